# revision 1
# baseline (speedup 1.0000x reference)
"""Trainium2 Bass kernel for nn_AttentionCrossLayer.

Math: in the reference, softmax over a length-1 axis is exactly 1.0, so
attn == v and q/k/wq/wk are dead code. With x0 the (never-mutated) input,
each layer's gate xw_i = out_i @ cw_i is a fixed linear function of x0:
    xw_i = x0 @ u_i + c_i,   u_i = Wv_i @ (Wo_i @ cw_i),
                             c_i = (bv_i @ Wo_i + bo_i) @ cw_i
and the layer recurrence x += x0 * xw_i + cb_i telescopes to
    out[b, d] = x0[b, d] * (x0[b, :] @ usum + cprime) + cbsum[d]
with usum = sum_i u_i  [D], cprime = 1 + sum_i c_i, cbsum = sum_i cb_i [D].

The tiny weight contractions happen host-side in float64. The device
kernel is one pass over x per core, 32 row-tiles of [128, 1024]:
  pass 1 (Vector): fused multiply + row-reduce (scalar_tensor_tensor
    with accum_out) -> per-row gate t. cprime rides in a constant
    column appended to x/u so the reduce emits the finished gate.
  pass 2: in-place x <- x * t + cbsum. When cbsum == 0 (the spec fills
    cb with zeros) this is a pure per-row scale, which the Scalar
    engine's activation op does with a per-partition scale AP — the
    Vector engine then only runs pass 1 and compute never gates the
    DMA stream. A general Vector-engine path handles cbsum != 0.

Loads issue from the sync engine (HWDGE) with a small outstanding cap
so the first tiles land quickly; stores issue from GpSimd (SWDGE). All
32 tiles stay SBUF-resident: no slot reuse, no WAR hazards, and one DMA
outstanding per semaphore (a multi-queue DMA increments its semaphore
in fractions of 16, so cumulative waits over a shared sem fire early).

Sharding: data-parallel over batch across 8 cores, weights replicated,
no cross-device comms.
"""

import numpy as np

L, B, D, H, K = 3, 32768, 1024, 8, 64
N_CORES = 8
B_LOC = B // N_CORES  # 4096 rows per core
P = 128
N_TILES = B_LOC // P  # 32
DP = D + 32  # slot stride 4224B = 128B aligned; col D holds the 1.0 constant
LOAD_CAP = 6  # max outstanding sync-engine loads

_cache = {}


def _build_program(cprime: float, zero_cb: bool):
    import concourse.bass as bass
    from concourse import mybir

    F32 = mybir.dt.float32
    MUL = mybir.AluOpType.mult
    ADD = mybir.AluOpType.add

    nc = bass.Bass()
    x = nc.declare_dram_parameter("x", [B_LOC, D], F32, isOutput=False)
    u = nc.declare_dram_parameter("u", [1, D], F32, isOutput=False)
    cb = nc.declare_dram_parameter("cb", [1, D], F32, isOutput=False)
    out = nc.declare_dram_parameter("out", [B_LOC, D], F32, isOutput=True)

    u_bcast = bass.AP(tensor=u.ap().tensor, offset=0, ap=[[0, P], [1, D]])
    cb_bcast = bass.AP(tensor=cb.ap().tensor, offset=0, ap=[[0, P], [1, D]])

    with (
        nc.sbuf_tensor([P, D + 1], F32) as ub,  # [:, :D]=usum, [:, D]=cprime
        nc.sbuf_tensor([P, D], F32) as cbb,
        nc.sbuf_tensor([P, N_TILES, DP], F32) as xt,  # [:, i, D] = 1.0
        nc.sbuf_tensor([P, 2, D + 1], F32) as oscr,  # throwaway STT main out
        nc.sbuf_tensor([P, N_TILES, 1], F32) as tsc,
        nc.semaphore("us") as us,
        nc.semaphore("cm") as cm,    # pass-1 reduces retired (Vector)
        nc.semaphore("cm2") as cm2,  # pass-2 writes retired
        nc.Block() as block,
    ):
        lds = [nc.alloc_semaphore(f"ld{i}") for i in range(N_TILES)]
        sts = [nc.alloc_semaphore(f"st{i}") for i in range(N_TILES)]

        # Loads run on the 8 HWDGE engines, stores on the 8 SWDGE
        # engines. Stores don't exist during the ramp, so hand the SWDGE
        # pool the first loads, and let the HWDGE pool (free first) take
        # the trailing stores.
        N_SW_LOADS = 3
        N_HW_STORES = 3

        @block.scalar
        def _(scalar):
            # broadcasts ride the scalar engine's DMA path so the first x
            # loads aren't queued behind them
            scalar.dma_start(out=ub[:, 0:D], in_=u_bcast).then_inc(us, 16)
            for i in (3, 4):
                scalar.dma_start(
                    out=xt[:, i, 0:D], in_=x[i * P : (i + 1) * P, :]
                ).then_inc(lds[i], 16)
            if not zero_cb:
                scalar.dma_start(out=cbb[:, :], in_=cb_bcast).then_inc(us, 16)
            else:
                # pass 2 on the Scalar engine: x <- x * t (cbsum == 0).
                # waiting on cm also orders us after Vector's memsets
                # (constant columns), which precede its first op1.
                for i in range(N_TILES):
                    scalar.wait_ge(cm, i + 1)
                    nc.scalar.mul(
                        out=xt[:, i, 0:D],
                        in_=xt[:, i, 0:D],
                        mul=tsc[:, i, :],
                    ).then_inc(cm2, 1)
                    if i >= N_TILES - N_HW_STORES:
                        # trailing stores on the (now idle) HWDGE pool;
                        # the self-wait makes the in-place mul retire
                        # before the DMA reads the tile
                        scalar.wait_ge(cm2, i + 1)
                        scalar.dma_start(
                            out=out[i * P : (i + 1) * P, :], in_=xt[:, i, 0:D]
                        ).then_inc(sts[i], 16)
                for i in range(N_TILES - N_HW_STORES, N_TILES):
                    scalar.wait_ge(sts[i], 16)

        @block.sync
        def _(sync):
            for i in range(N_SW_LOADS + 2, N_TILES):
                if i >= LOAD_CAP + N_SW_LOADS + 2:
                    sync.wait_ge(lds[i - LOAD_CAP], 16)
                sync.dma_start(
                    out=xt[:, i, 0:D], in_=x[i * P : (i + 1) * P, :]
                ).then_inc(lds[i], 16)

        @block.vector
        def _(vector):
            # constants: 1.0 column in every tile slot, cprime in ub,
            # zero activation bias
            nc.vector.memset(xt[:, :, D : D + 1], 1.0)
            nc.vector.memset(ub[:, D : D + 1], cprime)
            vector.wait_ge(us, 16 if zero_cb else 32)
            for i in range(N_TILES):
                vector.wait_ge(lds[i], 16)
                # oscr = x' * u' ; t_i = sum_free = x.usum + cprime
                nc.vector.scalar_tensor_tensor(
                    out=oscr[:, i % 2, :],
                    in0=xt[:, i, 0 : D + 1],
                    scalar=1.0,
                    in1=ub[:, :],
                    op0=MUL,
                    op1=MUL,
                    accum_out=tsc[:, i, :],
                ).then_inc(cm, 1)
                if not zero_cb:
                    # accumulator writeback must retire before t is read
                    vector.wait_ge(cm, i + 1)
                    # in place: x <- x * t + cbsum
                    nc.vector.scalar_tensor_tensor(
                        out=xt[:, i, 0:D],
                        in0=xt[:, i, 0:D],
                        scalar=tsc[:, i, :],
                        in1=cbb[:, :],
                        op0=MUL,
                        op1=ADD,
                    ).then_inc(cm2, 1)

        @block.gpsimd
        def _(gpsimd):
            for i in range(N_SW_LOADS):
                gpsimd.dma_start(
                    out=xt[:, i, 0:D], in_=x[i * P : (i + 1) * P, :]
                ).then_inc(lds[i], 16)
            n_store = N_TILES - (N_HW_STORES if zero_cb else 0)
            for i in range(n_store):
                gpsimd.wait_ge(cm2, i + 1)
                gpsimd.dma_start(
                    out=out[i * P : (i + 1) * P, :], in_=xt[:, i, 0:D]
                ).then_inc(sts[i], 16)
            for i in range(n_store):
                gpsimd.wait_ge(sts[i], 16)

    return nc


def _precompute(wv, bv, wo, bo, cw, cb):
    """Host-side f64 contraction of the small per-layer weights."""
    usum = np.zeros(D, np.float64)
    cprime = 1.0
    for i in range(L):
        Wv = wv[i].reshape(D, H * K).astype(np.float64)
        Wo = wo[i].reshape(H * K, D).astype(np.float64)
        cwi = cw[i].reshape(D).astype(np.float64)
        wocw = Wo @ cwi
        usum += Wv @ wocw
        cprime += float(bv[i].reshape(H * K).astype(np.float64) @ wocw)
        cprime += float(bo[i].astype(np.float64) @ cwi)
    cbsum = cb.astype(np.float64).sum(axis=0)
    return usum.astype(np.float32), float(np.float32(cprime)), cbsum.astype(np.float32)


def _ensure_trace_hook_importable():
    # bass_utils unconditionally imports antenv.axon_hooks when the
    # BASS_TRACE env var is set; some images lack that module. A None
    # hook makes bass_utils skip tracing gracefully.
    try:
        import antenv.axon_hooks  # noqa: F401
    except ImportError:
        import sys
        import types

        mod = types.ModuleType("antenv.axon_hooks")
        mod.get_axon_ntff_profile_hook = lambda: None
        mod.set_axon_ntff_profile_hook = lambda hook: None
        sys.modules["antenv.axon_hooks"] = mod


def kernel(x, wq, bq, wk, bk, wv, bv, wo, bo, cw, cb):
    from concourse.bass_utils import run_bass_kernel_spmd

    _ensure_trace_hook_importable()

    x = np.ascontiguousarray(np.asarray(x, dtype=np.float32))
    usum, cprime, cbsum = _precompute(
        np.asarray(wv), np.asarray(bv), np.asarray(wo), np.asarray(bo),
        np.asarray(cw), np.asarray(cb),
    )
    zero_cb = not np.any(cbsum)

    key = (cprime, zero_cb)
    if key not in _cache:
        _cache[key] = _build_program(cprime, zero_cb)
    nc = _cache[key]

    u2 = usum.reshape(1, D)
    cb2 = cbsum.reshape(1, D)
    in_maps = [
        {"x": x[c * B_LOC : (c + 1) * B_LOC], "u": u2, "cb": cb2}
        for c in range(N_CORES)
    ]
    res = run_bass_kernel_spmd(nc, in_maps, list(range(N_CORES)))
    return np.concatenate([res.results[c]["out"] for c in range(N_CORES)], axis=0)



# revision 2
# speedup vs baseline: 1.0343x; 1.0343x over previous
"""Trainium2 Bass kernel for nn_AttentionCrossLayer.

Math: in the reference, softmax over a length-1 axis is exactly 1.0, so
attn == v and q/k/wq/wk are dead code. With x0 the (never-mutated) input,
each layer's gate xw_i = out_i @ cw_i is a fixed linear function of x0:
    xw_i = x0 @ u_i + c_i,   u_i = Wv_i @ (Wo_i @ cw_i),
                             c_i = (bv_i @ Wo_i + bo_i) @ cw_i
and the layer recurrence x += x0 * xw_i + cb_i telescopes to
    out[b, d] = x0[b, d] * (x0[b, :] @ usum + cprime) + cbsum[d]
with usum = sum_i u_i  [D], cprime = 1 + sum_i c_i, cbsum = sum_i cb_i [D].

The tiny weight contractions happen host-side in float64. The device
kernel is one pass over x per core, 32 row-tiles of [128, 1024]:
  pass 1 (Vector): fused multiply + row-reduce (scalar_tensor_tensor
    with accum_out) -> per-row gate t. cprime rides in a constant
    column appended to x/u so the reduce emits the finished gate.
  pass 2: in-place x <- x * t + cbsum. When cbsum == 0 (the spec fills
    cb with zeros) this is a pure per-row scale on the Scalar engine
    (activation per-partition scale AP). A Vector path handles
    cbsum != 0.

Perf notes (from baseline trace analysis, core 0, all 8 cores live):
  - The SDMA pool sustains ~420 GB/s when fed by clean streaming rings.
    The old u-broadcast DMA (128 replicated latency-bound 4KB
    descriptors) poisoned the packet round-robin for the first ~16us
    (~315 GB/s). u now lands as a single 4KB row and is broadcast
    across partitions by a PE rank-1 matmul (ones[1,128]^T @ u[1,D]),
    costing ~1us of otherwise-idle Tensor time and no DMA pollution.
  - Loads split across both HWDGE rings (sync + scalar), issued
    back-to-back with no outstanding cap: all 32 tiles stay SBUF
    resident so there are no WAR hazards to throttle.
  - Stores ride SWDGE (gpsimd) incrementing ONE cumulative semaphore;
    the single final wait (>= 32*16) only fires when every store's
    last byte is confirmed in HBM.
  - Block(no_gpsimd_drain=True): skips the ~3.6us GpSimd dge_drain in
    the block epilogue; completion is already guaranteed by the
    store-semaphore wait.

Sharding: data-parallel over batch across 8 cores, weights replicated,
no cross-device comms.
"""

import numpy as np

L, B, D, H, K = 3, 32768, 1024, 8, 64
N_CORES = 8
B_LOC = B // N_CORES  # 4096 rows per core
P = 128
N_TILES = B_LOC // P  # 32
DP = D + 32  # slot stride 4224B = 128B aligned; col D holds the 1.0 constant

_cache = {}


def _build_program(cprime: float, zero_cb: bool):
    import concourse.bass as bass
    from concourse import mybir

    F32 = mybir.dt.float32
    MUL = mybir.AluOpType.mult
    ADD = mybir.AluOpType.add

    nc = bass.Bass()
    x = nc.declare_dram_parameter("x", [B_LOC, D], F32, isOutput=False)
    u = nc.declare_dram_parameter("u", [1, D], F32, isOutput=False)
    cb = nc.declare_dram_parameter("cb", [1, D], F32, isOutput=False)
    out = nc.declare_dram_parameter("out", [B_LOC, D], F32, isOutput=True)

    cb_bcast = bass.AP(tensor=cb.ap().tensor, offset=0, ap=[[0, P], [1, D]])

    # tile ownership: gpsimd primes 0-2 (SWDGE ring is otherwise idle
    # early), scalar takes 3,4 then the odd tiles, sync the even tiles.
    # Interleaving keeps delivery roughly in consumption order while
    # both HWDGE rings stay busy.
    gp_tiles = [0, 1, 2]
    sc_tiles = [3, 4] + [i for i in range(5, N_TILES) if i % 2 == 1]
    sy_tiles = [i for i in range(5, N_TILES) if i % 2 == 0]

    with (
        nc.sbuf_tensor([P, D + 1], F32) as ub,  # [:, :D]=usum, [:, D]=cprime
        nc.sbuf_tensor([1, P], F32) as ones,  # matmul stationary
        nc.sbuf_tensor([1, D], F32) as u1,  # usum row, partition 0
        nc.sbuf_tensor([P, D], F32) as cbb,
        nc.sbuf_tensor([P, N_TILES, DP], F32) as xt,  # [:, i, D] = 1.0
        nc.sbuf_tensor([P, 2, D + 1], F32) as oscr,  # throwaway STT main out
        nc.sbuf_tensor([P, N_TILES, 1], F32) as tsc,
        nc.psum_tensor([P, D], F32) as pub,  # u broadcast via PE
        nc.semaphore("us") as us,  # u row landed
        nc.semaphore("vr") as vr,  # ones memset retired (Vector)
        nc.semaphore("mm") as mm,  # PE broadcast done
        nc.semaphore("cbs") as cbs,  # cb broadcast landed (general path)
        nc.semaphore("cm") as cm,  # pass-1 reduces retired (Vector)
        nc.semaphore("cm2") as cm2,  # pass-2 writes retired
        nc.semaphore("st") as st,  # cumulative store completions
        nc.Block(no_gpsimd_drain=True) as block,
    ):
        lds = [nc.alloc_semaphore(f"ld{i}") for i in range(N_TILES)]

        @block.scalar
        def _(scalar):
            # 4KB row load first on the scalar HWDGE ring: lands fast,
            # unblocks the PE broadcast while tile loads stream.
            scalar.dma_start(out=u1[:, :], in_=u.ap()).then_inc(us, 16)
            if not zero_cb:
                scalar.dma_start(out=cbb[:, :], in_=cb_bcast).then_inc(cbs, 16)
            for i in sc_tiles:
                scalar.dma_start(
                    out=xt[:, i, 0:D], in_=x[i * P : (i + 1) * P, :]
                ).then_inc(lds[i], 16)
            if zero_cb:
                # pass 2 on the Scalar engine: x <- x * t (cbsum == 0).
                for i in range(N_TILES):
                    scalar.wait_ge(cm, i + 1)
                    nc.scalar.mul(
                        out=xt[:, i, 0:D],
                        in_=xt[:, i, 0:D],
                        mul=tsc[:, i, :],
                    ).then_inc(cm2, 1)

        @block.sync
        def _(sync):
            for i in sy_tiles:
                sync.dma_start(
                    out=xt[:, i, 0:D], in_=x[i * P : (i + 1) * P, :]
                ).then_inc(lds[i], 16)

        @block.tensor
        def _(tensor):
            # broadcast u across partitions: pub[p, d] = ones[p] * u1[d]
            tensor.wait_ge(vr, 1)
            tensor.wait_ge(us, 16)
            nc.tensor.matmul(
                pub[:, 0:512], ones[:, :], u1[:, 0:512], start=True, stop=True
            )
            nc.tensor.matmul(
                pub[:, 512:D], ones[:, :], u1[:, 512:D], start=True, stop=True
            ).then_inc(mm, 1)

        @block.vector
        def _(vector):
            nc.vector.memset(ones[:, :], 1.0).then_inc(vr, 1)
            nc.vector.memset(xt[:, :, D : D + 1], 1.0)
            nc.vector.memset(ub[:, D : D + 1], cprime)
            vector.wait_ge(mm, 1)
            nc.vector.tensor_copy(ub[:, 0:D], pub[:, :])
            if not zero_cb:
                vector.wait_ge(cbs, 16)
            for i in range(N_TILES):
                vector.wait_ge(lds[i], 16)
                # oscr = x' * u' ; t_i = sum_free = x.usum + cprime
                nc.vector.scalar_tensor_tensor(
                    out=oscr[:, i % 2, :],
                    in0=xt[:, i, 0 : D + 1],
                    scalar=1.0,
                    in1=ub[:, :],
                    op0=MUL,
                    op1=MUL,
                    accum_out=tsc[:, i, :],
                ).then_inc(cm, 1)
                if not zero_cb:
                    # accumulator writeback must retire before t is read
                    vector.wait_ge(cm, i + 1)
                    # in place: x <- x * t + cbsum
                    nc.vector.scalar_tensor_tensor(
                        out=xt[:, i, 0:D],
                        in0=xt[:, i, 0:D],
                        scalar=tsc[:, i, :],
                        in1=cbb[:, :],
                        op0=MUL,
                        op1=ADD,
                    ).then_inc(cm2, 1)

        @block.gpsimd
        def _(gpsimd):
            for i in gp_tiles:
                gpsimd.dma_start(
                    out=xt[:, i, 0:D], in_=x[i * P : (i + 1) * P, :]
                ).then_inc(lds[i], 16)
            for i in range(N_TILES):
                gpsimd.wait_ge(cm2, i + 1)
                gpsimd.dma_start(
                    out=out[i * P : (i + 1) * P, :], in_=xt[:, i, 0:D]
                ).then_inc(st, 16)
            gpsimd.wait_ge(st, 16 * N_TILES)

    return nc


def _precompute(wv, bv, wo, bo, cw, cb):
    """Host-side f64 contraction of the small per-layer weights."""
    usum = np.zeros(D, np.float64)
    cprime = 1.0
    for i in range(L):
        Wv = wv[i].reshape(D, H * K).astype(np.float64)
        Wo = wo[i].reshape(H * K, D).astype(np.float64)
        cwi = cw[i].reshape(D).astype(np.float64)
        wocw = Wo @ cwi
        usum += Wv @ wocw
        cprime += float(bv[i].reshape(H * K).astype(np.float64) @ wocw)
        cprime += float(bo[i].astype(np.float64) @ cwi)
    cbsum = cb.astype(np.float64).sum(axis=0)
    return usum.astype(np.float32), float(np.float32(cprime)), cbsum.astype(np.float32)


def _ensure_trace_hook_importable():
    # bass_utils unconditionally imports antenv.axon_hooks when the
    # BASS_TRACE env var is set; some images lack that module. A None
    # hook makes bass_utils skip tracing gracefully.
    try:
        import antenv.axon_hooks  # noqa: F401
    except ImportError:
        import sys
        import types

        mod = types.ModuleType("antenv.axon_hooks")
        mod.get_axon_ntff_profile_hook = lambda: None
        mod.set_axon_ntff_profile_hook = lambda hook: None
        sys.modules["antenv.axon_hooks"] = mod


def kernel(x, wq, bq, wk, bk, wv, bv, wo, bo, cw, cb):
    from concourse.bass_utils import run_bass_kernel_spmd

    _ensure_trace_hook_importable()

    x = np.ascontiguousarray(np.asarray(x, dtype=np.float32))
    usum, cprime, cbsum = _precompute(
        np.asarray(wv), np.asarray(bv), np.asarray(wo), np.asarray(bo),
        np.asarray(cw), np.asarray(cb),
    )
    zero_cb = not np.any(cbsum)

    key = (cprime, zero_cb)
    if key not in _cache:
        _cache[key] = _build_program(cprime, zero_cb)
    nc = _cache[key]

    u2 = usum.reshape(1, D)
    cb2 = cbsum.reshape(1, D)
    in_maps = [
        {"x": x[c * B_LOC : (c + 1) * B_LOC], "u": u2, "cb": cb2}
        for c in range(N_CORES)
    ]
    res = run_bass_kernel_spmd(nc, in_maps, list(range(N_CORES)))
    return np.concatenate([res.results[c]["out"] for c in range(N_CORES)], axis=0)


# revision 4
# speedup vs baseline: 1.0723x; 1.0367x over previous
"""Trainium2 Bass kernel for nn_AttentionCrossLayer.

Math: in the reference, softmax over a length-1 axis is exactly 1.0, so
attn == v and q/k/wq/wk are dead code. With x0 the (never-mutated) input,
each layer's gate xw_i = out_i @ cw_i is a fixed linear function of x0:
    xw_i = x0 @ u_i + c_i,   u_i = Wv_i @ (Wo_i @ cw_i),
                             c_i = (bv_i @ Wo_i + bo_i) @ cw_i
and the layer recurrence x += x0 * xw_i + cb_i telescopes to
    out[b, d] = x0[b, d] * (x0[b, :] @ usum + cprime) + cbsum[d]
with usum = sum_i u_i  [D], cprime = 1 + sum_i c_i, cbsum = sum_i cb_i [D].

The tiny weight contractions happen host-side in float64. The device
kernel is one pass over x per core, 32 row-tiles of [128, 1024]:
  pass 1 (Vector): fused multiply + row-reduce (scalar_tensor_tensor
    with accum_out) -> per-row gate t. cprime rides in a constant
    column appended to x/u so the reduce emits the finished gate.
  pass 2: in-place x <- x * t + cbsum. When cbsum == 0 (the spec fills
    cb with zeros) this is a pure per-row scale on the Scalar engine
    (activation per-partition scale AP). A Vector path handles
    cbsum != 0.

Perf notes (from baseline trace analysis, core 0, all 8 cores live):
  - The SDMA pool sustains ~420 GB/s when fed by clean streaming rings.
    The old u-broadcast DMA (128 replicated latency-bound 4KB
    descriptors) poisoned the packet round-robin for the first ~16us
    (~315 GB/s). u now lands as a single 4KB row and is broadcast
    across partitions by a PE rank-1 matmul (ones[1,128]^T @ u[1,D]),
    costing ~1us of otherwise-idle Tensor time and no DMA pollution.
  - Loads split across both HWDGE rings (sync + scalar), issued
    back-to-back with no outstanding cap: all 32 tiles stay SBUF
    resident so there are no WAR hazards to throttle.
  - Stores ride SWDGE (gpsimd) incrementing ONE cumulative semaphore;
    the single final wait (>= 32*16) only fires when every store's
    last byte is confirmed in HBM.
  - Block(no_gpsimd_drain=True): skips the ~3.6us GpSimd dge_drain in
    the block epilogue; completion is already guaranteed by the
    store-semaphore wait.

Sharding: data-parallel over batch across 8 cores, weights replicated,
no cross-device comms.
"""

import numpy as np

L, B, D, H, K = 3, 32768, 1024, 8, 64
N_CORES = 8
B_LOC = B // N_CORES  # 4096 rows per core
P = 128
N_TILES = B_LOC // P  # 32
DP = D + 32  # slot stride 4224B = 128B aligned; col D holds the 1.0 constant

_cache = {}


def _build_program(cprime: float, zero_cb: bool):
    import concourse.bass as bass
    from concourse import mybir

    F32 = mybir.dt.float32
    MUL = mybir.AluOpType.mult
    ADD = mybir.AluOpType.add

    nc = bass.Bass()
    x = nc.declare_dram_parameter("x", [B_LOC, D], F32, isOutput=False)
    u = nc.declare_dram_parameter("u", [1, D], F32, isOutput=False)
    cb = nc.declare_dram_parameter("cb", [1, D], F32, isOutput=False)
    out = nc.declare_dram_parameter("out", [B_LOC, D], F32, isOutput=True)

    cb_bcast = bass.AP(tensor=cb.ap().tensor, offset=0, ap=[[0, P], [1, D]])

    # Tile ownership. The SWDGE (gpsimd) ring self-paces and sustains
    # the full ~420 GB/s alone, while an HWDGE ring serializes its DMAs
    # (~150-190 GB/s) and BLOCKS the issuing engine once the ring backs
    # up. So: gpsimd streams tiles 0..23 in consumption order plus all
    # stores (FIFO: loads drain first, then stores — clean read phase
    # then write phase); sync carries the tail tiles 24..31 on its
    # otherwise-idle HWDGE ring (delivered early, consumed last);
    # scalar stays lean (u row + pass-2) so cm2 — which gates stores —
    # advances at load rate.
    gp_tiles = list(range(0, 24))
    sy_tiles = list(range(24, N_TILES))
    sc_tiles = []

    with (
        nc.sbuf_tensor([P, D + 1], F32) as ub,  # [:, :D]=usum, [:, D]=cprime
        nc.sbuf_tensor([1, P], F32) as ones,  # matmul stationary
        nc.sbuf_tensor([1, D], F32) as u1,  # usum row, partition 0
        nc.sbuf_tensor([P, D], F32) as cbb,
        nc.sbuf_tensor([P, N_TILES, DP], F32) as xt,  # [:, i, D] = 1.0
        nc.sbuf_tensor([P, 2, D + 1], F32) as oscr,  # throwaway STT main out
        nc.sbuf_tensor([P, N_TILES, 1], F32) as tsc,
        nc.psum_tensor([P, D], F32) as pub,  # u broadcast via PE
        nc.semaphore("us") as us,  # u row landed
        nc.semaphore("vr") as vr,  # ones memset retired (Vector)
        nc.semaphore("mm") as mm,  # PE broadcast done
        nc.semaphore("cbs") as cbs,  # cb broadcast landed (general path)
        nc.semaphore("cm") as cm,  # pass-1 reduces retired (Vector)
        nc.semaphore("cm2") as cm2,  # pass-2 writes retired
        nc.semaphore("st") as st,  # cumulative store completions
        nc.Block(no_gpsimd_drain=True) as block,
    ):
        lds = [nc.alloc_semaphore(f"ld{i}") for i in range(N_TILES)]

        @block.scalar
        def _(scalar):
            # 4KB row load first on the scalar HWDGE ring: lands fast,
            # unblocks the PE broadcast while tile loads stream.
            scalar.dma_start(out=u1[:, :], in_=u.ap()).then_inc(us, 16)
            if not zero_cb:
                scalar.dma_start(out=cbb[:, :], in_=cb_bcast).then_inc(cbs, 16)
            for i in sc_tiles:
                scalar.dma_start(
                    out=xt[:, i, 0:D], in_=x[i * P : (i + 1) * P, :]
                ).then_inc(lds[i], 16)
            if zero_cb:
                # pass 2 on the Scalar engine: x <- x * t (cbsum == 0).
                for i in range(N_TILES):
                    scalar.wait_ge(cm, i + 1)
                    nc.scalar.mul(
                        out=xt[:, i, 0:D],
                        in_=xt[:, i, 0:D],
                        mul=tsc[:, i, :],
                    ).then_inc(cm2, 1)

        @block.sync
        def _(sync):
            for i in sy_tiles:
                sync.dma_start(
                    out=xt[:, i, 0:D], in_=x[i * P : (i + 1) * P, :]
                ).then_inc(lds[i], 16)

        @block.tensor
        def _(tensor):
            # broadcast u across partitions: pub[p, d] = ones[p] * u1[d]
            tensor.wait_ge(vr, 1)
            tensor.wait_ge(us, 16)
            nc.tensor.matmul(
                pub[:, 0:512], ones[:, :], u1[:, 0:512], start=True, stop=True
            )
            nc.tensor.matmul(
                pub[:, 512:D], ones[:, :], u1[:, 512:D], start=True, stop=True
            ).then_inc(mm, 1)

        @block.vector
        def _(vector):
            nc.vector.memset(ones[:, :], 1.0).then_inc(vr, 1)
            nc.vector.memset(xt[:, :, D : D + 1], 1.0)
            nc.vector.memset(ub[:, D : D + 1], cprime)
            vector.wait_ge(mm, 1)
            nc.vector.tensor_copy(ub[:, 0:D], pub[:, :])
            if not zero_cb:
                vector.wait_ge(cbs, 16)
            for i in range(N_TILES):
                vector.wait_ge(lds[i], 16)
                # oscr = x' * u' ; t_i = sum_free = x.usum + cprime
                nc.vector.scalar_tensor_tensor(
                    out=oscr[:, i % 2, :],
                    in0=xt[:, i, 0 : D + 1],
                    scalar=1.0,
                    in1=ub[:, :],
                    op0=MUL,
                    op1=MUL,
                    accum_out=tsc[:, i, :],
                ).then_inc(cm, 1)
                if not zero_cb:
                    # accumulator writeback must retire before t is read
                    vector.wait_ge(cm, i + 1)
                    # in place: x <- x * t + cbsum
                    nc.vector.scalar_tensor_tensor(
                        out=xt[:, i, 0:D],
                        in0=xt[:, i, 0:D],
                        scalar=tsc[:, i, :],
                        in1=cbb[:, :],
                        op0=MUL,
                        op1=ADD,
                    ).then_inc(cm2, 1)

        @block.gpsimd
        def _(gpsimd):
            for i in gp_tiles:
                gpsimd.dma_start(
                    out=xt[:, i, 0:D], in_=x[i * P : (i + 1) * P, :]
                ).then_inc(lds[i], 16)
            for i in range(N_TILES):  # stores queue after loads in the ring
                gpsimd.wait_ge(cm2, i + 1)
                gpsimd.dma_start(
                    out=out[i * P : (i + 1) * P, :], in_=xt[:, i, 0:D]
                ).then_inc(st, 16)
            gpsimd.wait_ge(st, 16 * N_TILES)

    return nc


def _precompute(wv, bv, wo, bo, cw, cb):
    """Host-side f64 contraction of the small per-layer weights."""
    usum = np.zeros(D, np.float64)
    cprime = 1.0
    for i in range(L):
        Wv = wv[i].reshape(D, H * K).astype(np.float64)
        Wo = wo[i].reshape(H * K, D).astype(np.float64)
        cwi = cw[i].reshape(D).astype(np.float64)
        wocw = Wo @ cwi
        usum += Wv @ wocw
        cprime += float(bv[i].reshape(H * K).astype(np.float64) @ wocw)
        cprime += float(bo[i].astype(np.float64) @ cwi)
    cbsum = cb.astype(np.float64).sum(axis=0)
    return usum.astype(np.float32), float(np.float32(cprime)), cbsum.astype(np.float32)


def _ensure_trace_hook_importable():
    # bass_utils unconditionally imports antenv.axon_hooks when the
    # BASS_TRACE env var is set; some images lack that module. A None
    # hook makes bass_utils skip tracing gracefully.
    try:
        import antenv.axon_hooks  # noqa: F401
    except ImportError:
        import sys
        import types

        mod = types.ModuleType("antenv.axon_hooks")
        mod.get_axon_ntff_profile_hook = lambda: None
        mod.set_axon_ntff_profile_hook = lambda hook: None
        sys.modules["antenv.axon_hooks"] = mod


def kernel(x, wq, bq, wk, bk, wv, bv, wo, bo, cw, cb):
    from concourse.bass_utils import run_bass_kernel_spmd

    _ensure_trace_hook_importable()

    x = np.ascontiguousarray(np.asarray(x, dtype=np.float32))
    usum, cprime, cbsum = _precompute(
        np.asarray(wv), np.asarray(bv), np.asarray(wo), np.asarray(bo),
        np.asarray(cw), np.asarray(cb),
    )
    zero_cb = not np.any(cbsum)

    key = (cprime, zero_cb)
    if key not in _cache:
        _cache[key] = _build_program(cprime, zero_cb)
    nc = _cache[key]

    u2 = usum.reshape(1, D)
    cb2 = cbsum.reshape(1, D)
    in_maps = [
        {"x": x[c * B_LOC : (c + 1) * B_LOC], "u": u2, "cb": cb2}
        for c in range(N_CORES)
    ]
    res = run_bass_kernel_spmd(nc, in_maps, list(range(N_CORES)))
    return np.concatenate([res.results[c]["out"] for c in range(N_CORES)], axis=0)
